# revision 6
# baseline (speedup 1.0000x reference)
"""AdditiveAttentionLayer Trainium2 kernel (v6: mixed fp8-DR / bf16).

Math: logits[t,s,b] = scores[s,b] (masked s<t) are t-independent, so
softmax-attention collapses to exclusive prefix sums along T:
    context[t] = (sum_{s<t} e^{scores[s]} * x[s]) / (sum_{s<t} e^{scores[s]})

Precision split (measured error budget): the out-GEMM X-part dominates the
signal, so it stays bf16; the hp GEMM (softmax washes quantization out) and
the ctx-part (small relative magnitude) run fp8e4 DoubleRow at 2x rate.

Per-core (batch-sharded, 4 of 32 batches):
  1. XTb = bf16(8*X^T) via PE transposes + ACT-copy(scale=8); XT8 =
     fp8(XTb) via gpsimd TT (idle engine); hp = tanh(X@W) via fp8 DR
     (XT8 pairs x W8 pairs, W8 = fp8(16W)), ACT Tanh(scale=1/128);
     scores via fused DVE scalar_tensor_tensor (mul + accum reduce)
     against a partition-broadcast proj.
  2. w = exp(scores) col [128, NT]; z = strict-prefix via masked matmul +
     carry rank-1 matmul; izb = [128, T] broadcast of 4/z
     (transpose-to-psum-row + K=1 matmul broadcast).
  3. Prefix P^T[h,t]: one [128x130] masked matmul per h-chunk, 2 chunks
     share a PSUM bank (start=T/F), running carry S added and izb scale
     applied in ONE fused DVE scalar_tensor_tensor -> ptz8 fp8 (= 4*ctx);
     S updated from the in-mask totals column, one strided add per pair.
  4. out = tanh(X@Wc1.T + ctx@Wc2.T): 8 bf16 matmuls (XTb x WcT1b =
     bf16(8*Wc1^T)) + 4 fp8 DR (ptz8 x WcT8f = fp8(16*Wc2^T)) per
     n-half into one PSUM group; single ACT Tanh(scale=1/64).
  Rows 0..1 patched to inputs on host.
"""

import sys
from contextlib import ExitStack

import numpy as np

if "/opt/trn_rl_repo" not in sys.path:
    sys.path.insert(0, "/opt/trn_rl_repo")

import concourse.bass as bass
import concourse.mybir as mybir
from concourse.bacc import Bacc
from concourse.bass_utils import run_bass_kernel_spmd
from concourse.masks import make_identity, make_upper_triangular
from concourse.tile import TileContext

T = 1024
B_FULL = 32
NCORES = 8
BB = B_FULL // NCORES  # 4 batches per core
H = 1024
KC = H // 128  # 8 contraction chunks
NP = KC // 2  # 4 fp8 DoubleRow chunk-pairs
NT = T // 128  # 8 t-tiles

F32 = mybir.dt.float32
BF16 = mybir.dt.bfloat16
F8 = mybir.dt.float8e4
AF = mybir.ActivationFunctionType
ALU = mybir.AluOpType
DR = mybir.MatmulPerfMode.DoubleRow

SX = 8.0  # X scale (both XTb bf16 and XT8 fp8 carry 8*X)
SW = 16.0  # W / Wc2 fp8 scale
SW1 = 8.0  # Wc1 bf16 scale
S_HP = 1.0 / (SX * SW)  # 1/128, hp psum descale
S_OUT = 1.0 / (SX * SW1)  # 1/64, out psum descale (= 4*16 on ctx side)


def build():
    nc = Bacc()

    x_d = nc.dram_tensor("inputs", [T, BB, H], F32, kind="ExternalInput")
    w_d = nc.dram_tensor("W", [H, H], F32, kind="ExternalInput")
    p_d = nc.dram_tensor("proj", [H], F32, kind="ExternalInput")
    cw_d = nc.dram_tensor("concat_w", [H, 2 * H], F32, kind="ExternalInput")
    out_d = nc.dram_tensor("out", [T, BB, H], F32, kind="ExternalOutput")

    with ExitStack() as es:
        tc = es.enter_context(TileContext(nc))

        # ---------------- pools ----------------
        cpool = es.enter_context(tc.tile_pool(name="consts", bufs=1))
        wstg = es.enter_context(tc.tile_pool(name="wstg", bufs=2))
        xfp = es.enter_context(tc.tile_pool(name="xf", bufs=10))
        xtbp = es.enter_context(tc.tile_pool(name="xtb", bufs=2))
        xt8p = es.enter_context(tc.tile_pool(name="xt8", bufs=2))
        wxp = es.enter_context(tc.tile_pool(name="wx", bufs=16))
        hprp = es.enter_context(tc.tile_pool(name="hpr", bufs=2))
        scrp = es.enter_context(tc.tile_pool(name="scr", bufs=1))
        smp = es.enter_context(tc.tile_pool(name="sm", bufs=2))
        izbp = es.enter_context(tc.tile_pool(name="izb", bufs=2))
        ptz8p = es.enter_context(tc.tile_pool(name="ptz8", bufs=2))
        op = es.enter_context(tc.tile_pool(name="osb", bufs=2))

        # PSUM (8 banks): tpps 2 (X/Wc transposes + izrow), hpps 2 (hp
        # groups + proj/izb broadcasts + tot/carry), ptzp 2 (prefix pairs
        # + z group), apsp 2 (out groups).
        tpps = es.enter_context(tc.tile_pool(name="tpps", bufs=2, space="PSUM"))
        hpps = es.enter_context(tc.tile_pool(name="hpps", bufs=2, space="PSUM"))
        ptzp = es.enter_context(tc.tile_pool(name="ptzp", bufs=2, space="PSUM"))
        apsp = es.enter_context(tc.tile_pool(name="apsp", bufs=2, space="PSUM"))

        # ---------------- constants ----------------
        id_sb = cpool.tile([128, 128], F32, name="id_sb")
        make_identity(nc, id_sb)
        id_b16 = cpool.tile([128, 128], BF16, name="id_b16")
        nc.vector.tensor_copy(id_b16, id_sb)

        mraw = cpool.tile([128, 130], F32, name="mraw")
        nc.gpsimd.memset(mraw, 0.0)
        make_upper_triangular(nc, mraw[:, 0:128], val=1.0, diag=False)
        u8_sb = cpool.tile([128, 128], BF16, name="u8_sb")
        nc.vector.tensor_copy(u8_sb, mraw[:, 0:128])
        # prefix mask: strict upper + ones col 128 + zero col 129
        nc.gpsimd.memset(mraw[:, 128:129], 1.0)
        u8x_sb = cpool.tile([128, 130], BF16, name="u8x_sb")
        nc.vector.tensor_copy(u8x_sb, mraw)

        ones_f = cpool.tile([128, 130], F32, name="ones_f")
        nc.gpsimd.memset(ones_f, 1.0)
        ones_b = cpool.tile([128, 130], BF16, name="ones_b")
        nc.vector.tensor_copy(ones_b, ones_f)
        onesf_row = ones_f[0:1, 0:128]
        onesb_col = ones_b[:, 128:129]
        onesb_row = ones_b[0:1, 0:128]
        ones_128b = ones_b[:, 0:128]

        W8 = cpool.tile([128, KC, H], F8, name="W8")
        WcT1b = cpool.tile([128, KC, H], BF16, name="WcT1b")
        WcT8f = cpool.tile([128, KC, H], F8, name="WcT8f")
        proj_b = cpool.tile([128, H], BF16, name="proj_b")

        # ---------------- input staging (emission order = DMA priority) ----
        proj_row = cpool.tile([1, H], F32, name="proj_row")
        nc.sync.dma_start(proj_row, p_d.rearrange("(o k) -> o k", o=1))
        xfs = [[None] * NT for _ in range(BB)]

        def emit_xf_dma(j, fine):
            for tt in range(NT):
                xf = xfp.tile([128, H], F32, name="xf")
                if fine and tt < 2:
                    for q in range(4):
                        nc.sync.dma_start(
                            xf[:, q * 256 : (q + 1) * 256],
                            x_d[
                                tt * 128 : (tt + 1) * 128, j, q * 256 : (q + 1) * 256
                            ],
                        )
                else:
                    nc.sync.dma_start(xf, x_d[tt * 128 : (tt + 1) * 128, j, :])
                xfs[j][tt] = xf

        def emit_xf_dma_tiles(j, tts, fine):
            for tt in tts:
                xf = xfp.tile([128, H], F32, name="xf")
                if fine:
                    for q in range(4):
                        nc.sync.dma_start(
                            xf[:, q * 256 : (q + 1) * 256],
                            x_d[
                                tt * 128 : (tt + 1) * 128, j, q * 256 : (q + 1) * 256
                            ],
                        )
                else:
                    nc.sync.dma_start(xf, x_d[tt * 128 : (tt + 1) * 128, j, :])
                xfs[j][tt] = xf

        emit_xf_dma_tiles(0, [0, 1], fine=True)

        # proj -> partition-broadcast bf16 [128, H]
        for g in range(2):
            pb_ps = hpps.tile([128, 512], F32, name="hp_ps")
            nc.tensor.matmul(
                pb_ps,
                onesf_row,
                proj_row[0:1, g * 512 : (g + 1) * 512],
                start=True,
                stop=True,
                skip_group_check=True,
            )
            nc.vector.tensor_copy(proj_b[:, g * 512 : (g + 1) * 512], pb_ps)

        # W chunks -> W8 (fp8, x16)
        for c in range(KC):
            stg = wstg.tile([128, H], F32, name="stg")
            nc.sync.dma_start(stg, w_d[c * 128 : (c + 1) * 128, :])
            nc.vector.tensor_scalar_mul(W8[:, c, :], stg, SW)

        emit_xf_dma_tiles(0, list(range(2, NT)), fine=False)

        # cw staging DMAs (Wc1 half first)
        cw_stg = {}
        for half in range(2):
            for ro in range(8):
                stg = wstg.tile([128, H], F32, name="stg")
                base = half * H
                nc.sync.dma_start(
                    stg, cw_d[ro * 128 : (ro + 1) * 128, base : base + H]
                )
                cw_stg[(half, ro)] = stg

        def emit_wct_block(half, ro):
            stg = cw_stg[(half, ro)]
            for cg in range(2):
                tp = tpps.tile([128, 4, 128], F32, name="tp")
                for ci in range(4):
                    c = cg * 4 + ci
                    nc.tensor.transpose(
                        tp[:, ci, :], stg[:, c * 128 : (c + 1) * 128], id_sb
                    )
                dst = WcT1b if half == 0 else WcT8f
                nc.scalar.mul(
                    dst[:, cg * 4 : (cg + 1) * 4, ro * 128 : (ro + 1) * 128],
                    tp,
                    SW1 if half == 0 else SW,
                )

        wxs = [[None] * NT for _ in range(BB)]
        sms = [None] * BB

        def emit_phase1(j):
            """Transposes + XTb/XT8 + hp (fp8 DR) + scores + w + wx per tile."""
            XTb = xtbp.tile([128, KC, T], BF16, name="XTb")
            XT8 = xt8p.tile([128, KC, T], F8, name="XT8")
            scores_col = smp.tile([128, NT], F32, name="scores_col")
            w_colf = smp.tile([128, NT], F32, name="w_colf")
            w_col8 = smp.tile([128, NT], BF16, name="w_col8")
            S = smp.tile([128, KC], F32, name="S")
            nc.vector.memset(S, 0.0)
            sms[j] = (XTb, XT8, scores_col, w_colf, w_col8, S)
            for tt in range(NT):
                ttsl = slice(tt * 128, (tt + 1) * 128)
                xf = xfs[j][tt]
                for cg in range(2):
                    cgs = slice(cg * 4, (cg + 1) * 4)
                    tp = tpps.tile([128, 4, 128], F32, name="tp")
                    for ci in range(4):
                        c = cg * 4 + ci
                        nc.tensor.transpose(
                            tp[:, ci, :], xf[:, c * 128 : (c + 1) * 128], id_sb
                        )
                    nc.scalar.mul(XTb[:, cgs, ttsl], tp, SX)
                    nc.gpsimd.tensor_tensor(
                        XT8[:, cgs, ttsl],
                        XTb[:, cgs, ttsl],
                        ones_128b.unsqueeze(1).broadcast_to([128, 4, 128]),
                        ALU.mult,
                    )
                hpr = hprp.tile([128, H], BF16, name="hpr")
                for kh in range(2):
                    ksl = slice(kh * 512, (kh + 1) * 512)
                    hp_ps = hpps.tile([128, 512], F32, name="hp_ps")
                    for p in range(NP):
                        nc.tensor.matmul(
                            hp_ps,
                            XT8[:, 2 * p : 2 * p + 2, ttsl],
                            W8[:, 2 * p : 2 * p + 2, ksl],
                            start=(p == 0),
                            stop=(p == NP - 1),
                            perf_mode=DR,
                        )
                    nc.scalar.activation(hpr[:, ksl], hp_ps, AF.Tanh, scale=S_HP)
                scratch = scrp.tile([128, H], BF16, name="scratch")
                nc.vector.scalar_tensor_tensor(
                    scratch,
                    hpr,
                    1.0,
                    proj_b,
                    ALU.mult,
                    ALU.mult,
                    accum_out=scores_col[:, tt : tt + 1],
                )
                nc.scalar.activation(
                    w_colf[:, tt : tt + 1], scores_col[:, tt : tt + 1], AF.Exp
                )
                nc.vector.tensor_copy(w_col8[:, tt : tt + 1], w_colf[:, tt : tt + 1])
                wx = wxp.tile([128, H], F8, name="wx")
                nc.vector.tensor_scalar_mul(wx, xf, w_colf[:, tt : tt + 1])
                wxs[j][tt] = wx
                if tt == 3 and j + 1 < BB:
                    emit_xf_dma(j + 1, fine=False)
            if j == 0:
                for ro in range(8):
                    emit_wct_block(0, ro)

        def emit_phase234(j):
            XTb, XT8, scores_col, w_colf, w_col8, S = sms[j]

            # ---- z chain: z = strict-prefix of w (cross-tile carry) ----
            z_ps = ptzp.tile([128, NT], F32, name="pt_ps")
            nc.tensor.matmul(
                z_ps, u8_sb, w_col8, start=True, stop=False, skip_group_check=True
            )
            tot_ps = hpps.tile([8, 1], F32, name="hp_ps")
            nc.tensor.matmul(
                tot_ps, w_col8, onesb_col, start=True, stop=True,
                skip_group_check=True,
            )
            tot_col8 = smp.tile([8, 1], BF16, name="tot_col8")
            nc.vector.tensor_copy(tot_col8, tot_ps)
            carry_ps = hpps.tile([1, NT], F32, name="hp_ps")
            nc.tensor.matmul(
                carry_ps,
                tot_col8,
                u8_sb[0:8, 0:8],
                start=True,
                stop=True,
                skip_group_check=True,
            )
            carry_sb = smp.tile([1, NT], BF16, name="carry_sb")
            nc.vector.tensor_copy(carry_sb, carry_ps)
            nc.tensor.matmul(
                z_ps, onesb_row, carry_sb, start=False, stop=True,
                skip_group_check=True,
            )
            invz_colf = smp.tile([128, NT], F32, name="invz_colf")
            nc.vector.reciprocal(invz_colf, z_ps)
            iz4_col8 = smp.tile([128, NT], BF16, name="iz4_col8")
            nc.vector.tensor_scalar_mul(iz4_col8, invz_colf, 4.0)

            # izb = [128, T] broadcast of 4/z: transpose cols to one psum
            # row, then K=1 matmul broadcast.
            izrow_ps = tpps.tile([1, T], BF16, name="tp")
            for tt in range(NT):
                nc.tensor.matmul(
                    izrow_ps[0:1, tt * 128 : (tt + 1) * 128],
                    iz4_col8[:, tt : tt + 1],
                    id_b16,
                    is_transpose=True,
                    start=(tt == 0),
                    stop=(tt == NT - 1),
                    skip_group_check=True,
                )
            izrow_sb = smp.tile([1, T], BF16, name="izrow_sb")
            nc.scalar.copy(izrow_sb, izrow_ps)
            izb = izbp.tile([128, T], BF16, name="izb")
            for g in range(2):
                izb_ps = hpps.tile([128, 512], F32, name="hp_ps")
                nc.tensor.matmul(
                    izb_ps,
                    onesb_row,
                    izrow_sb[0:1, g * 512 : (g + 1) * 512],
                    start=True,
                    stop=True,
                    skip_group_check=True,
                )
                nc.vector.tensor_copy(izb[:, g * 512 : (g + 1) * 512], izb_ps)

            if j == 0:
                for ro in range(8):
                    emit_wct_block(1, ro)

            # ---- prefix + out GEMM per tile, interleaved so the PE
            # stays busy (X-part matmuls) while the DVE drains prefix
            # psums into ptz8 (STT) ----
            ptz8 = ptz8p.tile([128, KC, T], F8, name="ptz8")

            def emit_prefix_pair(tt, p, ttsl, wx):
                pt_ps = ptzp.tile([128, 2, 130], F32, name="pt_ps")
                for i in range(2):
                    c = 2 * p + i
                    nc.tensor.matmul(
                        pt_ps[:, i, :],
                        wx[:, c * 128 : (c + 1) * 128],
                        u8x_sb,
                        start=(i == 0),
                        stop=(i == 1),
                        skip_group_check=True,
                    )
                for i in range(2):
                    c = 2 * p + i
                    nc.vector.scalar_tensor_tensor(
                        ptz8[:, c, ttsl],
                        pt_ps[:, i, 0:128],
                        S[:, c : c + 1],
                        izb[:, ttsl],
                        ALU.add,
                        ALU.mult,
                    )
                nc.vector.tensor_add(
                    S[:, 2 * p : 2 * p + 2], S[:, 2 * p : 2 * p + 2],
                    pt_ps[:, 0:2, 128],
                )

            def emit_x_part(a_ps, ttsl, nsl):
                for k in range(KC):
                    nc.tensor.matmul(
                        a_ps,
                        XTb[:, k, ttsl],
                        WcT1b[:, k, nsl],
                        start=(k == 0),
                        stop=False,
                    )

            def emit_ctx_part(a_ps, ttsl, nsl):
                for p in range(NP):
                    nc.tensor.matmul(
                        a_ps,
                        ptz8[:, 2 * p : 2 * p + 2, ttsl],
                        WcT8f[:, 2 * p : 2 * p + 2, nsl],
                        start=False,
                        stop=(p == NP - 1),
                        perf_mode=DR,
                    )

            for tt in range(NT):
                ttsl = slice(tt * 128, (tt + 1) * 128)
                wx = wxs[j][tt]
                emit_prefix_pair(tt, 0, ttsl, wx)
                emit_prefix_pair(tt, 1, ttsl, wx)
                outsb = op.tile([128, H], F32, name="outsb")
                a_ps0 = apsp.tile([128, 512], F32, name="a_ps")
                emit_x_part(a_ps0, ttsl, slice(0, 512))
                emit_prefix_pair(tt, 2, ttsl, wx)
                emit_prefix_pair(tt, 3, ttsl, wx)
                emit_ctx_part(a_ps0, ttsl, slice(0, 512))
                nc.scalar.activation(outsb[:, 0:512], a_ps0, AF.Tanh, scale=S_OUT)
                a_ps1 = apsp.tile([128, 512], F32, name="a_ps")
                emit_x_part(a_ps1, ttsl, slice(512, 1024))
                emit_ctx_part(a_ps1, ttsl, slice(512, 1024))
                nc.scalar.activation(outsb[:, 512:1024], a_ps1, AF.Tanh, scale=S_OUT)
                nc.sync.dma_start(out_d[ttsl, j, :], outsb)

        for j in range(BB):
            emit_phase1(j)
            if j > 0:
                emit_phase234(j - 1)
        emit_phase234(BB - 1)

    nc.finalize()
    return nc


_NC = None


def _get_nc():
    global _NC
    if _NC is None:
        _NC = build()
    return _NC


def kernel(**inputs):
    x = np.ascontiguousarray(np.asarray(inputs["inputs"], dtype=np.float32))
    W = np.ascontiguousarray(np.asarray(inputs["W"], dtype=np.float32))
    proj = np.ascontiguousarray(np.asarray(inputs["proj"], dtype=np.float32))
    cw = np.ascontiguousarray(np.asarray(inputs["concat_w"], dtype=np.float32))

    nc = _get_nc()
    in_maps = [
        {
            "inputs": np.ascontiguousarray(x[:, i * BB : (i + 1) * BB, :]),
            "W": W,
            "proj": proj,
            "concat_w": cw,
        }
        for i in range(NCORES)
    ]
    res = run_bass_kernel_spmd(nc, in_maps, core_ids=list(range(NCORES)))
    out = np.concatenate([m["out"] for m in res.results], axis=1)
    out[:2] = x[:2]
    return out


# revision 8
# speedup vs baseline: 1.2207x; 1.2207x over previous
"""AdditiveAttentionLayer Trainium2 kernel (v8: merged-pipeline, mixed fp8/bf16).

Math: logits[t,s,b] = scores[s,b] (masked s<t) are t-independent, so
softmax-attention collapses to exclusive prefix sums along T:
    context[t] = (sum_{s<t} e^{scores[s]} * x[s]) / (sum_{s<t} e^{scores[s]})

Precision split (measured error budget): the out-GEMM X-part dominates the
signal, so it stays bf16; the hp GEMM (softmax washes quantization out) and
the ctx-part (small relative magnitude) run fp8e4 DoubleRow at 2x rate.

Schedule: phase-1 of batch j+1 (transpose/hp/scores) is interleaved per-tile
into phases-3/4 of batch j (prefix/out) so the PE stream stays dense (the PE
clock drops to a lower p-state after idle gaps) and ACT/DVE work hides under
PE-heavy sections.

Per-core (batch-sharded, 4 of 32 batches):
  1. xfb = bf16(X) (ACT cast after DMA); XTb = bf16(8*X^T) via bf16 PE
     transposes + ACT-copy(scale=8); XT8 = fp8(XTb) via gpsimd TT;
     hp = tanh(X@W) via fp8 DR (XT8 pairs x W8 pairs, W8 = fp8(16W)),
     ACT Tanh(scale=1/128); scores via fused DVE scalar_tensor_tensor.
  2. w = exp(scores) col [128, NT]; z = strict-prefix via masked matmul +
     carry rank-1 matmul; izb = [128, T] broadcast of 4/z
     (transpose-to-psum-row + K=1 matmul broadcast).
  3. Prefix P^T[h,t]: one [128x130] masked matmul per h-chunk, 2 chunks
     share a PSUM bank (start=T/F), running carry S added and izb scale
     applied in ONE fused DVE scalar_tensor_tensor -> ptz8 fp8 (= 4*ctx);
     S updated from the in-mask totals column, one strided add per pair.
  4. out = tanh(X@Wc1.T + ctx@Wc2.T): 8 bf16 matmuls (XTb x WcT1b =
     bf16(8*Wc1^T)) + 4 fp8 DR (ptz8 x WcT8f = fp8(16*Wc2^T)) per
     n-half into one PSUM group; single ACT Tanh(scale=1/64).
  Rows 0..1 patched to inputs on host.
"""

import sys
from contextlib import ExitStack

import numpy as np

if "/opt/trn_rl_repo" not in sys.path:
    sys.path.insert(0, "/opt/trn_rl_repo")

import concourse.bass as bass
import concourse.mybir as mybir
from concourse.bacc import Bacc
from concourse.bass_utils import run_bass_kernel_spmd
from concourse.masks import make_identity, make_upper_triangular
from concourse.tile import TileContext

T = 1024
B_FULL = 32
NCORES = 8
BB = B_FULL // NCORES  # 4 batches per core
H = 1024
KC = H // 128  # 8 contraction chunks
NP = KC // 2  # 4 fp8 DoubleRow chunk-pairs
NT = T // 128  # 8 t-tiles

F32 = mybir.dt.float32
BF16 = mybir.dt.bfloat16
F8 = mybir.dt.float8e4
AF = mybir.ActivationFunctionType
ALU = mybir.AluOpType
DR = mybir.MatmulPerfMode.DoubleRow

SX = 8.0  # X scale (both XTb bf16 and XT8 fp8 carry 8*X)
SW = 16.0  # W / Wc2 fp8 scale
SW1 = 8.0  # Wc1 bf16 scale
S_HP = 1.0 / (SX * SW)  # 1/128, hp psum descale
S_OUT = 1.0 / (SX * SW1)  # 1/64, out psum descale (= 4*16 on ctx side)


def build():
    nc = Bacc()

    x_d = nc.dram_tensor("inputs", [T, BB, H], F32, kind="ExternalInput")
    w_d = nc.dram_tensor("W", [H, H], F32, kind="ExternalInput")
    p_d = nc.dram_tensor("proj", [H], F32, kind="ExternalInput")
    cw_d = nc.dram_tensor("concat_w", [H, 2 * H], F32, kind="ExternalInput")
    out_d = nc.dram_tensor("out", [T, BB, H], F32, kind="ExternalOutput")

    with ExitStack() as es:
        tc = es.enter_context(TileContext(nc))

        # ---------------- pools ----------------
        cpool = es.enter_context(tc.tile_pool(name="consts", bufs=1))
        wstg = es.enter_context(tc.tile_pool(name="wstg", bufs=2))
        xfp = es.enter_context(tc.tile_pool(name="xf", bufs=4))
        xfbp = es.enter_context(tc.tile_pool(name="xfb", bufs=16))
        xtbp = es.enter_context(tc.tile_pool(name="xtb", bufs=2))
        xt8p = es.enter_context(tc.tile_pool(name="xt8", bufs=2))
        wxp = es.enter_context(tc.tile_pool(name="wx", bufs=12))
        hprp = es.enter_context(tc.tile_pool(name="hpr", bufs=2))
        scrp = es.enter_context(tc.tile_pool(name="scr", bufs=1))
        smp = es.enter_context(tc.tile_pool(name="sm", bufs=2))
        izbp = es.enter_context(tc.tile_pool(name="izb", bufs=2))
        ptz8p = es.enter_context(tc.tile_pool(name="ptz8", bufs=2))
        op = es.enter_context(tc.tile_pool(name="osb", bufs=2))

        # PSUM (8 banks): tpps 2 (X/Wc transposes + izrow), hpps 2 (hp
        # groups + proj/izb broadcasts + tot/carry), ptzp 2 (prefix pairs
        # + z group), apsp 2 (out groups).
        tpps = es.enter_context(tc.tile_pool(name="tpps", bufs=2, space="PSUM"))
        hpps = es.enter_context(tc.tile_pool(name="hpps", bufs=2, space="PSUM"))
        ptzp = es.enter_context(tc.tile_pool(name="ptzp", bufs=2, space="PSUM"))
        apsp = es.enter_context(tc.tile_pool(name="apsp", bufs=2, space="PSUM"))

        # ---------------- constants ----------------
        id_sb = cpool.tile([128, 128], F32, name="id_sb")
        make_identity(nc, id_sb)
        id_b16 = cpool.tile([128, 128], BF16, name="id_b16")
        nc.vector.tensor_copy(id_b16, id_sb)

        mraw = cpool.tile([128, 130], F32, name="mraw")
        nc.gpsimd.memset(mraw, 0.0)
        make_upper_triangular(nc, mraw[:, 0:128], val=1.0, diag=False)
        u8_sb = cpool.tile([128, 128], BF16, name="u8_sb")
        nc.vector.tensor_copy(u8_sb, mraw[:, 0:128])
        # prefix mask: strict upper + ones col 128 + zero col 129
        nc.gpsimd.memset(mraw[:, 128:129], 1.0)
        u8x_sb = cpool.tile([128, 130], BF16, name="u8x_sb")
        nc.vector.tensor_copy(u8x_sb, mraw)

        ones_f = cpool.tile([128, 130], F32, name="ones_f")
        nc.gpsimd.memset(ones_f, 1.0)
        ones_b = cpool.tile([128, 130], BF16, name="ones_b")
        nc.vector.tensor_copy(ones_b, ones_f)
        onesf_row = ones_f[0:1, 0:128]
        onesb_col = ones_b[:, 128:129]
        onesb_row = ones_b[0:1, 0:128]
        ones_128b = ones_b[:, 0:128]

        W8 = cpool.tile([128, KC, H], F8, name="W8")
        WcT1b = cpool.tile([128, KC, H], BF16, name="WcT1b")
        WcT8f = cpool.tile([128, KC, H], F8, name="WcT8f")
        proj_b = cpool.tile([128, H], BF16, name="proj_b")

        # ---------------- input staging ----------------
        proj_row = cpool.tile([1, H], F32, name="proj_row")
        nc.sync.dma_start(proj_row, p_d.rearrange("(o k) -> o k", o=1))
        xfbs = [[None] * NT for _ in range(BB)]

        def emit_xfb(j, tt, fine=False):
            """DMA one x tile (f32) and cast to bf16 on ACT."""
            xf = xfp.tile([128, H], F32, name="xf")
            if fine:
                for q in range(4):
                    nc.sync.dma_start(
                        xf[:, q * 256 : (q + 1) * 256],
                        x_d[tt * 128 : (tt + 1) * 128, j, q * 256 : (q + 1) * 256],
                    )
            else:
                nc.sync.dma_start(xf, x_d[tt * 128 : (tt + 1) * 128, j, :])
            xfb = xfbp.tile([128, H], BF16, name="xfb")
            nc.scalar.copy(xfb, xf)
            xfbs[j][tt] = xfb

        emit_xfb(0, 0, fine=True)
        emit_xfb(0, 1, fine=True)

        # proj -> partition-broadcast bf16 [128, H]
        for g in range(2):
            pb_ps = hpps.tile([128, 512], F32, name="hp_ps")
            nc.tensor.matmul(
                pb_ps,
                onesf_row,
                proj_row[0:1, g * 512 : (g + 1) * 512],
                start=True,
                stop=True,
                skip_group_check=True,
            )
            nc.vector.tensor_copy(proj_b[:, g * 512 : (g + 1) * 512], pb_ps)

        # W chunks -> W8 (fp8, x16)
        for c in range(KC):
            stg = wstg.tile([128, H], F32, name="stg")
            nc.sync.dma_start(stg, w_d[c * 128 : (c + 1) * 128, :])
            nc.vector.tensor_scalar_mul(W8[:, c, :], stg, SW)

        for tt in range(2, NT):
            emit_xfb(0, tt)

        # cw staging DMAs (Wc1 half first)
        cw_stg = {}
        for half in range(2):
            for ro in range(8):
                stg = wstg.tile([128, H], F32, name="stg")
                base = half * H
                nc.sync.dma_start(
                    stg, cw_d[ro * 128 : (ro + 1) * 128, base : base + H]
                )
                cw_stg[(half, ro)] = stg

        def emit_wct_block(half, ro):
            stg = cw_stg[(half, ro)]
            for cg in range(2):
                tp = tpps.tile([128, 4, 128], F32, name="tp")
                for ci in range(4):
                    c = cg * 4 + ci
                    nc.tensor.transpose(
                        tp[:, ci, :], stg[:, c * 128 : (c + 1) * 128], id_sb
                    )
                dst = WcT1b if half == 0 else WcT8f
                nc.scalar.mul(
                    dst[:, cg * 4 : (cg + 1) * 4, ro * 128 : (ro + 1) * 128],
                    tp,
                    SW1 if half == 0 else SW,
                )

        wxs = [[None] * NT for _ in range(BB)]
        sms = [None] * BB

        def alloc_sm(j):
            XTb = xtbp.tile([128, KC, T], BF16, name="XTb")
            XT8 = xt8p.tile([128, KC, T], F8, name="XT8")
            scores_col = smp.tile([128, NT], F32, name="scores_col")
            w_colf = smp.tile([128, NT], F32, name="w_colf")
            w_col8 = smp.tile([128, NT], BF16, name="w_col8")
            S = smp.tile([128, KC], F32, name="S")
            nc.vector.memset(S, 0.0)
            sms[j] = (XTb, XT8, scores_col, w_colf, w_col8, S)

        def emit_ph1_tile(j, tt):
            """Transposes + XTb/XT8 + hp (fp8 DR) + scores + w + wx."""
            XTb, XT8, scores_col, w_colf, w_col8, S = sms[j]
            ttsl = slice(tt * 128, (tt + 1) * 128)
            xfb = xfbs[j][tt]
            for cg in range(2):
                cgs = slice(cg * 4, (cg + 1) * 4)
                tp = tpps.tile([128, 4, 128], BF16, name="tp")
                for ci in range(4):
                    c = cg * 4 + ci
                    nc.tensor.transpose(
                        tp[:, ci, :], xfb[:, c * 128 : (c + 1) * 128], id_b16
                    )
                nc.scalar.mul(XTb[:, cgs, ttsl], tp, SX)
                nc.gpsimd.tensor_tensor(
                    XT8[:, cgs, ttsl],
                    XTb[:, cgs, ttsl],
                    ones_128b.unsqueeze(1).broadcast_to([128, 4, 128]),
                    ALU.mult,
                )
            hpr = hprp.tile([128, H], BF16, name="hpr")
            for kh in range(2):
                ksl = slice(kh * 512, (kh + 1) * 512)
                hp_ps = hpps.tile([128, 512], F32, name="hp_ps")
                for p in range(NP):
                    nc.tensor.matmul(
                        hp_ps,
                        XT8[:, 2 * p : 2 * p + 2, ttsl],
                        W8[:, 2 * p : 2 * p + 2, ksl],
                        start=(p == 0),
                        stop=(p == NP - 1),
                        perf_mode=DR,
                    )
                nc.scalar.activation(hpr[:, ksl], hp_ps, AF.Tanh, scale=S_HP)
            scratch = scrp.tile([128, H], BF16, name="scratch")
            nc.vector.scalar_tensor_tensor(
                scratch,
                hpr,
                1.0,
                proj_b,
                ALU.mult,
                ALU.mult,
                accum_out=scores_col[:, tt : tt + 1],
            )
            nc.scalar.activation(
                w_colf[:, tt : tt + 1], scores_col[:, tt : tt + 1], AF.Exp
            )
            nc.vector.tensor_copy(w_col8[:, tt : tt + 1], w_colf[:, tt : tt + 1])
            wx = wxp.tile([128, H], BF16, name="wx")
            nc.vector.tensor_scalar_mul(wx, xfb, w_colf[:, tt : tt + 1])
            wxs[j][tt] = wx

        # ---------------- batch-0 phase 1 (standalone; startup) ----------
        alloc_sm(0)
        for tt in range(NT):
            emit_ph1_tile(0, tt)
            emit_xfb(1, tt)
        for ro in range(8):
            emit_wct_block(0, ro)
        for ro in range(8):
            emit_wct_block(1, ro)

        # ---------------- merged loop: ph234(j) + ph1(j+1) ----------------
        for j in range(BB):
            XTb, XT8, scores_col, w_colf, w_col8, S = sms[j]
            last = j + 1 >= BB
            if not last:
                alloc_sm(j + 1)

            # ---- z chain part A ----
            z_ps = ptzp.tile([128, NT], F32, name="pt_ps")
            nc.tensor.matmul(
                z_ps, u8_sb, w_col8, start=True, stop=False, skip_group_check=True
            )
            tot_ps = hpps.tile([8, 1], F32, name="hp_ps")
            nc.tensor.matmul(
                tot_ps, w_col8, onesb_col, start=True, stop=True,
                skip_group_check=True,
            )
            tot_col8 = smp.tile([8, 1], BF16, name="tot_col8")
            nc.vector.tensor_copy(tot_col8, tot_ps)
            carry_ps = hpps.tile([1, NT], F32, name="hp_ps")
            nc.tensor.matmul(
                carry_ps,
                tot_col8,
                u8_sb[0:8, 0:8],
                start=True,
                stop=True,
                skip_group_check=True,
            )
            carry_sb = smp.tile([1, NT], BF16, name="carry_sb")
            nc.vector.tensor_copy(carry_sb, carry_ps)
            nc.tensor.matmul(
                z_ps, onesb_row, carry_sb, start=False, stop=True,
                skip_group_check=True,
            )
            invz_colf = smp.tile([128, NT], F32, name="invz_colf")
            nc.vector.reciprocal(invz_colf, z_ps)
            iz4_col8 = smp.tile([128, NT], BF16, name="iz4_col8")
            nc.vector.tensor_scalar_mul(iz4_col8, invz_colf, 4.0)

            ptz8 = ptz8p.tile([128, KC, T], F8, name="ptz8")

            def prefix_mm(tt, p, wx):
                pt_ps = ptzp.tile([128, 2, 130], F32, name="pt_ps")
                for i in range(2):
                    c = 2 * p + i
                    nc.tensor.matmul(
                        pt_ps[:, i, :],
                        wx[:, c * 128 : (c + 1) * 128],
                        u8x_sb,
                        start=(i == 0),
                        stop=(i == 1),
                        skip_group_check=True,
                    )
                return pt_ps

            def prefix_drain(tt, p, pt_ps, izb, ttsl):
                for i in range(2):
                    c = 2 * p + i
                    nc.vector.scalar_tensor_tensor(
                        ptz8[:, c, ttsl],
                        pt_ps[:, i, 0:128],
                        S[:, c : c + 1],
                        izb[:, ttsl],
                        ALU.add,
                        ALU.mult,
                    )
                nc.vector.tensor_add(
                    S[:, 2 * p : 2 * p + 2], S[:, 2 * p : 2 * p + 2],
                    pt_ps[:, 0:2, 128],
                )

            def emit_x_part(a_ps, ttsl, nsl):
                for k in range(KC):
                    nc.tensor.matmul(
                        a_ps,
                        XTb[:, k, ttsl],
                        WcT1b[:, k, nsl],
                        start=(k == 0),
                        stop=False,
                    )

            def emit_ctx_part(a_ps, ttsl, nsl):
                for p in range(NP):
                    nc.tensor.matmul(
                        a_ps,
                        ptz8[:, 2 * p : 2 * p + 2, ttsl],
                        WcT8f[:, 2 * p : 2 * p + 2, nsl],
                        start=False,
                        stop=(p == NP - 1),
                        perf_mode=DR,
                    )

            # ---- tile 0: interleave izb build under X-part matmuls ----
            t0sl = slice(0, 128)
            wx0 = wxs[j][0]
            pt0 = prefix_mm(0, 0, wx0)
            pt1 = prefix_mm(0, 1, wx0)
            outsb = op.tile([128, H], F32, name="outsb")
            a_ps0 = apsp.tile([128, 512], F32, name="a_ps")
            emit_x_part(a_ps0, t0sl, slice(0, 512))

            # izb = [128, T] broadcast of 4/z
            izrow_ps = tpps.tile([1, T], BF16, name="tp")
            for tt in range(NT):
                nc.tensor.matmul(
                    izrow_ps[0:1, tt * 128 : (tt + 1) * 128],
                    iz4_col8[:, tt : tt + 1],
                    id_b16,
                    is_transpose=True,
                    start=(tt == 0),
                    stop=(tt == NT - 1),
                    skip_group_check=True,
                )
            izrow_sb = smp.tile([1, T], BF16, name="izrow_sb")
            nc.scalar.copy(izrow_sb, izrow_ps)
            izb = izbp.tile([128, T], BF16, name="izb")
            for g in range(2):
                izb_ps = hpps.tile([128, 512], F32, name="hp_ps")
                nc.tensor.matmul(
                    izb_ps,
                    onesb_row,
                    izrow_sb[0:1, g * 512 : (g + 1) * 512],
                    start=True,
                    stop=True,
                    skip_group_check=True,
                )
                nc.vector.tensor_copy(izb[:, g * 512 : (g + 1) * 512], izb_ps)

            prefix_drain(0, 0, pt0, izb, t0sl)
            prefix_drain(0, 1, pt1, izb, t0sl)
            pt2 = prefix_mm(0, 2, wx0)
            prefix_drain(0, 2, pt2, izb, t0sl)
            pt3 = prefix_mm(0, 3, wx0)
            prefix_drain(0, 3, pt3, izb, t0sl)
            emit_ctx_part(a_ps0, t0sl, slice(0, 512))
            nc.scalar.activation(outsb[:, 0:512], a_ps0, AF.Tanh, scale=S_OUT)
            a_ps1 = apsp.tile([128, 512], F32, name="a_ps")
            emit_x_part(a_ps1, t0sl, slice(512, 1024))
            emit_ctx_part(a_ps1, t0sl, slice(512, 1024))
            nc.scalar.activation(outsb[:, 512:1024], a_ps1, AF.Tanh, scale=S_OUT)
            nc.sync.dma_start(out_d[t0sl, j, :], outsb)
            if not last:
                emit_ph1_tile(j + 1, 0)
                emit_xfb(j + 2, 0) if j + 2 < BB else None

            # ---- tiles 1..7 ----
            for tt in range(1, NT):
                ttsl = slice(tt * 128, (tt + 1) * 128)
                wx = wxs[j][tt]
                pta = prefix_mm(tt, 0, wx)
                prefix_drain(tt, 0, pta, izb, ttsl)
                ptb = prefix_mm(tt, 1, wx)
                prefix_drain(tt, 1, ptb, izb, ttsl)
                outsb = op.tile([128, H], F32, name="outsb")
                a_ps0 = apsp.tile([128, 512], F32, name="a_ps")
                emit_x_part(a_ps0, ttsl, slice(0, 512))
                ptc = prefix_mm(tt, 2, wx)
                prefix_drain(tt, 2, ptc, izb, ttsl)
                ptd = prefix_mm(tt, 3, wx)
                prefix_drain(tt, 3, ptd, izb, ttsl)
                emit_ctx_part(a_ps0, ttsl, slice(0, 512))
                nc.scalar.activation(outsb[:, 0:512], a_ps0, AF.Tanh, scale=S_OUT)
                a_ps1 = apsp.tile([128, 512], F32, name="a_ps")
                emit_x_part(a_ps1, ttsl, slice(512, 1024))
                emit_ctx_part(a_ps1, ttsl, slice(512, 1024))
                nc.scalar.activation(
                    outsb[:, 512:1024], a_ps1, AF.Tanh, scale=S_OUT
                )
                nc.sync.dma_start(out_d[ttsl, j, :], outsb)
                if not last:
                    emit_ph1_tile(j + 1, tt)
                    if j + 2 < BB:
                        emit_xfb(j + 2, tt)

    nc.finalize()
    return nc


_NC = None


def _get_nc():
    global _NC
    if _NC is None:
        _NC = build()
    return _NC


def kernel(**inputs):
    x = np.ascontiguousarray(np.asarray(inputs["inputs"], dtype=np.float32))
    W = np.ascontiguousarray(np.asarray(inputs["W"], dtype=np.float32))
    proj = np.ascontiguousarray(np.asarray(inputs["proj"], dtype=np.float32))
    cw = np.ascontiguousarray(np.asarray(inputs["concat_w"], dtype=np.float32))

    nc = _get_nc()
    in_maps = [
        {
            "inputs": np.ascontiguousarray(x[:, i * BB : (i + 1) * BB, :]),
            "W": W,
            "proj": proj,
            "concat_w": cw,
        }
        for i in range(NCORES)
    ]
    res = run_bass_kernel_spmd(nc, in_maps, core_ids=list(range(NCORES)))
    out = np.concatenate([m["out"] for m in res.results], axis=1)
    out[:2] = x[:2]
    return out
